# revision 25
# baseline (speedup 1.0000x reference)
"""Trainium2 Bass kernel for nn_AttentionManifold (SPD manifold attention).

For each of bs*m=2048 SPD matrices X (100x100): Q/K/V = W^T X W (64x64),
logQ/K/V = matrix log, log-Euclidean attention, mixed = prob-weighted sum
of logV, out = matrix exp(mixed).

Matrix log: Newton-Schulz coupled sqrt chain, L=3 levels, with
first-order level corrections  log A = 2^L log Y_L - sum_l 2^l log W_l,
log W ~= -(I - W);  series log(Y) via deg-8/12 Paterson-Stockmeyer.

Chain scheme '2s_approx' (emulator-validated): exact-transpose pairs
(Y, Yt, Z, Zt) in fp16, P = aI + bW only (no Pt):
    W    = {lhsT=Zt_h, rhs=Y_h}              (per matrix)
    P    = aI + b psW                        (stt)
    Y'   = {lhsT=Yt_h, rhs=P_h}              = Y P
    [Yt'|Z'] = {lhsT=bd(P), rhs=[Yt|Z]}      = [P^T Yt | P^T Z]  (packed)
    Zt'  = {lhsT=Z_h, rhs=P_h}               = (P^T Z)^T bitwise
Every W is a congruence of the SPD input => fp16-stable.

exp via scaling-squaring (deg-4 Horner, 4 squarings).
Sharding: pure data parallelism, bs=32 -> 4 samples per NeuronCore.
"""
import numpy as np
from contextlib import ExitStack

C_NORM = 16.0
BS, M, DIN, DOUT = 32, 64, 100, 64
NCORES = 8
NSAMP = BS // NCORES
NP_PAIR = M // 2          # 32 pairs per sample
PB = 4                    # pairs per chain batch
NBATCH = NP_PAIR // PB    # 8

SCHED_V = [
    [(8.965874126, -13.460097634), (2.380408822, -0.250737931),
     (2.380408822, -0.250532192), (2.380408822, -0.250326648),
     (0.861964497, -0.071654452), (1.542284382, -0.519941516)],
    [(7.758850039, -8.666077201), (0.987610378, -0.093162713),
     (1.645967366, -0.5826622), (1.507505828, -0.502426376)],
    [(6.551825952, -5.22018671), (0.65339645, -0.038866921)],
]
SCHED_QK = [
    [(8.965874126, -13.460097634), (2.380408822, -0.250737931),
     (2.380408822, -0.250532192), (2.380408822, -0.250326648),
     (0.861964497, -0.071654452), (1.542284382, -0.519941516)],
    [(7.758850039, -8.666077201), (0.987610378, -0.093162713),
     (1.645967366, -0.5826622)],
    [(6.404040404, -4.899837718), (0.670769231, -0.04132838)],
]
DEG_QK = 8
DEG_V = 8
EXP_DEG = 4
EXP_SQ = 4
L = 3
WBS = PB * 64             # 256
SBW = PB * 256            # 1024


def emit_kernel(nc, tc, ctx, x_ap, wq_ap, wk_ap, wv_ap, out_ap, nsamp=NSAMP,
                taps=None):
    def tap(name, t):
        if taps is not None and name in taps:
            nc.sync.dma_start(out=taps[name], in_=t)

    import concourse.mybir as mybir
    from concourse.bass import ds, ts
    from concourse.masks import make_identity

    f32 = mybir.dt.float32
    f32r = mybir.dt.float32r
    f16 = mybir.dt.float16
    AX = mybir.AxisListType
    OP = mybir.AluOpType
    ACT = mybir.ActivationFunctionType

    # ---- engine rotation helpers ----
    _rrc = [0]
    _rrs = [0]

    from concourse.bass import MemorySpace as _MS

    def _psum(*aps):
        return any(a.space == _MS.PSUM for a in aps)

    def rr_copy(out, in_, scale=None):
        pool = ((nc.vector, nc.scalar) if _psum(out, in_)
                else (nc.vector, nc.scalar, nc.gpsimd))
        e = pool[_rrc[0] % len(pool)]
        _rrc[0] += 1
        if e is nc.scalar:
            nc.scalar.activation(out=out, in_=in_, func=ACT.Copy, bias=0.0,
                                 scale=1.0 if scale is None else float(scale))
        elif scale is None:
            e.tensor_copy(out=out, in_=in_)
        else:
            e.tensor_scalar_mul(out, in_, float(scale))

    def rr_stt(out, in0, scalar, in1):
        nc.vector.scalar_tensor_tensor(out=out, in0=in0, scalar=float(scalar),
                                       in1=in1, op0=OP.mult, op1=OP.add)

    # ---------------- pools ----------------
    const = ctx.enter_context(tc.tile_pool(name="const", bufs=1))
    work = ctx.enter_context(tc.tile_pool(name="work", bufs=2))
    big = ctx.enter_context(tc.tile_pool(name="big", bufs=1))
    chain = ctx.enter_context(tc.tile_pool(name="chain", bufs=2))
    ps_w = ctx.enter_context(tc.tile_pool(name="ps_w", bufs=1, space="PSUM"))
    ps_bc = ctx.enter_context(tc.tile_pool(name="ps_bc", bufs=2, space="PSUM"))
    ps_a = ctx.enter_context(tc.tile_pool(name="ps_a", bufs=1, space="PSUM"))
    ps_b = ctx.enter_context(tc.tile_pool(name="ps_b", bufs=1, space="PSUM"))

    # ---------------- constants ----------------
    W3f = const.tile([DIN, 3 * DOUT], f32)
    nc.sync.dma_start(out=W3f[:, 0:DOUT], in_=wq_ap)
    nc.sync.dma_start(out=W3f[:, DOUT:2 * DOUT], in_=wk_ap)
    nc.sync.dma_start(out=W3f[:, 2 * DOUT:3 * DOUT], in_=wv_ap)
    W3r = const.tile([DIN, 256], f32r)
    nc.vector.memset(W3r.bitcast(f32), 0.0)
    nc.vector.tensor_copy(out=W3r[:, 0:192], in_=W3f)
    WQKh = const.tile([DIN, 128], f16)        # f16 weights for Q/K stage-2
    nc.vector.tensor_copy(out=WQKh, in_=W3f[:, 0:128])

    IREP = const.tile([128, 64], f16)
    make_identity(nc, IREP[0:64, :])
    make_identity(nc, IREP[64:128, :])

    aI = {}

    def get_aI(val):
        val = float(val)
        if val not in aI:
            t = const.tile([128, WBS], f16, tag=f"aI{len(aI)}",
                           name=f"aI{len(aI)}")
            for p in range(PB):
                nc.vector.tensor_scalar_mul(t[:, ts(p, 64)], IREP, val)
            aI[val] = t
        return aI[val]

    for lv in SCHED_QK + SCHED_V:
        for a, b in lv:
            get_aI(a)
    for c0 in (1.0, 0.25, 0.125):
        get_aI(c0)
    I7f = const.tile([128, WBS], f32)
    for p in range(PB):
        nc.vector.tensor_scalar_mul(I7f[:, ts(p, 64)], IREP, float(2 ** L - 1))
    cI_mm = {}

    def get_cI(val):
        val = float(val)
        if val not in cI_mm:
            t = const.tile([128, 64], f16, tag=f"cImm{len(cI_mm)}",
                           name=f"cImm{len(cI_mm)}")
            nc.vector.tensor_scalar_mul(t, IREP, val)
            cI_mm[val] = t
        return cI_mm[val]

    for v in (1.0, 0.5, 1.0 / 3.0, 2.0, 3.0):
        get_cI(v)
    for lv in SCHED_QK + SCHED_V:
        for a, b in lv:
            get_cI(a / b)
    for v in (0.25, 0.2, 1.0 / 6, 1.0 / 7, 0.125, 1.0 / 9, 0.1,
              1.0 / 11, 1.0 / 12):
        get_cI(v)

    ones_col = const.tile([64, 1], f32)
    nc.vector.memset(ones_col, 1.0)
    ones_col_h = const.tile([64, 1], f16)
    nc.vector.memset(ones_col_h, 16.0)    # folds 1/16 exp prescale into inv
    ones_row = const.tile([1, 64], f32)
    nc.vector.memset(ones_row, 1.0)
    bias_ln = const.tile([64, 1], f32)
    nc.vector.memset(bias_ln, 1.0 + 64e-6)

    # DRAM scratch for partition-moving transposes (DRAM APs unconstrained)
    scrV = nc.dram_tensor("scrV", [64, M * 64], f16, kind="Internal").ap()
    scrM = nc.dram_tensor("scrM", [64, M * 64], f16, kind="Internal").ap()


    def r3(t):
        """[p, (n c)] -> [p, n, 64] view"""
        return t.rearrange("p (n c) -> p n c", c=64)

    def slot(S, f):
        """S [128, (PB,4,64)] -> slot view [128, PB, 64]"""
        return S.rearrange("p (n four c) -> p n four c", four=4, c=64)[:, :, f, :]

    # =====================================================================
    def chain_gen(cn, sched, deg, init_t, b, flat_t):
        """One chain batch: NS chain + corrections + series -> flat_t."""
        ib = r3(init_t[:, ds(b * PB * 64, PB * 64)])     # [128, PB, 64]
        S_cur = None
        Y = Yt = Z = Zt = None       # [128, PB, 64] views
        adj = False
        ACC = None

        for l in range(L):
            steps = sched[l]
            for j, (a, bc) in enumerate(steps):
                Pd = chain.tile([128, WBS], f16, tag=f"Pd{cn}", name=f"Pd{cn}")
                Pd3 = r3(Pd)
                aIt = get_aI(a)
                if j == 0:
                    src = ib if l == 0 else Y
                    rr_stt(Pd3, src, bc, r3(aIt))
                    yield
                else:
                    psW = ps_w.tile([128, WBS], f32, tag="psW", name="psW")
                    psW3 = r3(psW)
                    abI = get_cI(a / bc)
                    Ipat = get_aI(1.0)
                    for h in (0, 1):
                        hs = slice(h * 64, h * 64 + 64)
                        nc.tensor.matmul(psW[hs, :], abI[hs, :], Ipat[hs, :],
                                         start=True, stop=False)
                    for p in range(PB):
                        for h in (0, 1):
                            hs = slice(h * 64, h * 64 + 64)
                            nc.tensor.matmul(psW3[hs, p], Zt[hs, p], Y[hs, p],
                                             start=False,
                                             stop=(p == PB - 1),
                                             skip_group_check=True)
                    yield
                    nc.vector.tensor_scalar_mul(Pd, psW, float(bc))
                    yield
                # ---- updates ----
                psBC = ps_bc.tile([128, SBW], f32, tag="psBC", name="psBC")
                ps4 = psBC.rearrange("p (n four c) -> p n four c", four=4, c=64)
                YtP = ib if (l == 0 and j == 0) else Yt
                for p in range(PB):
                    for h in (0, 1):
                        hs = slice(h * 64, h * 64 + 64)
                        # Y' = Yt^T P ; Yt' = P^T Yt
                        nc.tensor.matmul(ps4[hs, p, 0, :], YtP[hs, p],
                                         Pd3[hs, p], start=True, stop=True)
                        nc.tensor.matmul(ps4[hs, p, 1, :], Pd3[hs, p],
                                         YtP[hs, p], start=True, stop=True)
                        if j > 0:
                            # Z' = P^T Z ; Zt' = Z^T P
                            nc.tensor.matmul(ps4[hs, p, 2, :], Pd3[hs, p],
                                             Z[hs, p], start=True, stop=True)
                            nc.tensor.matmul(ps4[hs, p, 3, :], Z[hs, p],
                                             Pd3[hs, p], start=True, stop=True)
                yield
                S_new = chain.tile([128, SBW], f16, tag=f"S{cn}", name=f"S{cn}")
                s4 = S_new.rearrange("p (n four c) -> p n four c", four=4, c=64)
                if j == 0:
                    nc.vector.tensor_copy(out=s4[:, :, 0:2, :],
                                          in_=ps4[:, :, 0:2, :])
                    Z = Pd3
                    Zt = Pd3
                    adj = False
                else:
                    nc.scalar.activation(out=S_new[:, 0:SBW * 3 // 4],
                                         in_=psBC[:, 0:SBW * 3 // 4],
                                         func=ACT.Copy, bias=0.0, scale=1.0)
                    nc.vector.tensor_copy(out=S_new[:, SBW * 3 // 4:],
                                          in_=psBC[:, SBW * 3 // 4:])
                    Z = slot(S_new, 2)
                    Zt = slot(S_new, 3)
                    adj = True
                Y = slot(S_new, 0)
                Yt = slot(S_new, 1)
                S_cur = S_new
                yield
            # ---- level end correction: psWe = Zt^T Y + Y^T Zt ----
            psWe = ps_w.tile([128, WBS], f32, tag="psW", name="psWe")
            pw3 = r3(psWe)
            for p in range(PB):
                for h in (0, 1):
                    hs = slice(h * 64, h * 64 + 64)
                    nc.tensor.matmul(pw3[hs, p], Zt[hs, p], Y[hs, p],
                                     start=True, stop=False)
                    nc.tensor.matmul(pw3[hs, p], Y[hs, p], Zt[hs, p],
                                     start=False, stop=True)
            yield
            if l == 0:
                ACC = chain.tile([128, WBS], f32, tag=f"acc{cn}",
                                 name=f"acc{cn}")
                rr_stt(ACC, psWe, -0.5, I7f)
            else:
                rr_stt(ACC, psWe, -float(2 ** l) / 2.0, ACC)
            yield
        # ================= series =================
        E = chain.tile([128, WBS], f16, tag=f"E{cn}", name=f"E{cn}")
        rr_stt(r3(E), Y, -1.0, r3(get_aI(1.0)))
        yield
        powers = {1: E}
        for k, rt in ((2, 1), (3, 2), (4, 3)):
            psE = ps_a.tile([128, WBS], f32, tag="scrA", name="psE")
            pe3 = r3(psE)
            e1 = r3(powers[1])
            ert = r3(powers[rt])
            for p in range(PB):
                for h in (0, 1):
                    hs = slice(h * 64, h * 64 + 64)
                    nc.tensor.matmul(pe3[hs, p], e1[hs, p], ert[hs, p],
                                     start=True, stop=True)
            Ek = chain.tile([128, WBS], f16, tag=f"E{k}{cn}", name=f"E{k}{cn}")
            rr_copy(Ek, psE)
            powers[k] = Ek
            yield
        E2, E3, E4 = powers[2], powers[3], powers[4]

        def combo(coefs, dst_tag):
            """PE-accumulated c0 I + c1 E + c2 E2 + c3 E3 + c4 E4 -> f16."""
            psC = ps_a.tile([128, WBS], f32, tag="scrA", name="psC")
            ops = [(get_cI(coefs[0]), get_aI(1.0))] + [
                (get_cI(cv), pw) for cv, pw in
                zip(coefs[1:], (E, E2, E3, E4)) if cv]
            for i, (lh, rh) in enumerate(ops):
                for h in (0, 1):
                    hs = slice(h * 64, h * 64 + 64)
                    nc.tensor.matmul(psC[hs, :], lh[hs, :], rh[hs, :],
                                     start=(i == 0),
                                     stop=(i == len(ops) - 1),
                                     skip_group_check=(i > 0))
            Ct = chain.tile([128, WBS], f16, tag=dst_tag, name=dst_tag)
            rr_copy(Ct, psC)
            return Ct

        C = combo((0.25, 0.2, 1.0 / 6, 1.0 / 7, 0.125), f"C{cn}")
        yield
        if deg == 12:
            C2 = combo((0.125, 1.0 / 9, 0.1, 1.0 / 11, 1.0 / 12), f"C2{cn}")
            psH = ps_a.tile([128, WBS], f32, tag="scrA", name="psH")
            ph3, e43, c23 = r3(psH), r3(E4), r3(C2)
            for p in range(PB):
                for h in (0, 1):
                    hs = slice(h * 64, h * 64 + 64)
                    nc.tensor.matmul(ph3[hs, p], e43[hs, p], c23[hs, p],
                                     start=True, stop=True)
            yield
            CH = chain.tile([128, WBS], f16, tag=f"C2{cn}", name=f"CH{cn}")
            nc.vector.tensor_tensor(out=CH, in0=psH, in1=C, op=OP.add)
            C = CH
            yield
        # psB0 = 1*E + E4@C + (1/2)E2 + (1/3)E3   (accumulated group)
        psB0 = ps_a.tile([128, WBS], f32, tag="scrA", name="psB0")
        e43, c3 = r3(E4), r3(C)
        for h in (0, 1):
            hs = slice(h * 64, h * 64 + 64)
            nc.tensor.matmul(psB0[hs, :], get_cI(1.0)[hs, :], E[hs, :],
                             start=True, stop=False)
        for p in range(PB):
            for h in (0, 1):
                hs = slice(h * 64, h * 64 + 64)
                nc.tensor.matmul(r3(psB0)[hs, p], e43[hs, p], c3[hs, p],
                                 start=False, stop=False,
                                 skip_group_check=True)
        for h in (0, 1):
            hs = slice(h * 64, h * 64 + 64)
            nc.tensor.matmul(psB0[hs, :], get_cI(0.5)[hs, :], E2[hs, :],
                             start=False, stop=False, skip_group_check=True)
            nc.tensor.matmul(psB0[hs, :], get_cI(1.0 / 3.0)[hs, :], E3[hs, :],
                             start=False, stop=True, skip_group_check=True)
        yield
        # LS = -2^L psB0 + ACC -> flat (strided, per h)
        fl3 = flat_t.rearrange("p (pr two c) -> p pr two c", two=2, c=64)
        acc3 = r3(ACC)
        for h in (0, 1):
            hs = slice(h * 64, h * 64 + 64)
            rr_stt(fl3[:, ds(b * PB, PB), h, :], r3(psB0)[hs], -float(2 ** L),
                   acc3[hs])
        yield

    # ======================= per-sample pipeline =========================
    for s in range(nsamp):
        initQ = work.tile([128, NP_PAIR * 64], f16, tag="initQ", name="initQ")
        initK = work.tile([128, NP_PAIR * 64], f16, tag="initK", name="initK")
        initV = work.tile([128, NP_PAIR * 64], f16, tag="initV", name="initV")

        # ---------------- congruence ----------------
        for g in range(8):          # 8 matrices per group
            if g % 2 == 0:
                xbuf = work.tile([DIN, 16 * DIN], f32r, tag="xbuf", name="xbuf")
                nc.gpsimd.dma_start(
                    out=xbuf.rearrange("p (i c) -> p i c", c=DIN),
                    in_=x_ap[s, ds(g * 8, 16)].rearrange("i p c -> p i c"))
            pqks = []
            pvs = []
            for r in range(2):      # 2 rounds x 4 matrices
                ps1 = ps_b.tile([DIN, 4 * 256], f32, tag="scrB", name="ps1")
                for mi in range(4):
                    mg = (g % 2) * 8 + r * 4 + mi
                    nc.tensor.matmul(ps1[:, ts(mi, 256)],
                                     xbuf[:, ts(mg, DIN)], W3r,
                                     start=True, stop=True)
                pqk = work.tile([DIN, 4 * 128], f16, tag="pqk", name="pqk")
                pv = work.tile([DIN, 4 * 64], f32, tag="pv", name="pv")
                rr_copy(pqk.rearrange("p (n c) -> p n c", c=128),
                        ps1.rearrange("p (n c) -> p n c", c=256)[:, :, 0:128])
                rr_copy(pv.rearrange("p (n c) -> p n c", c=64),
                        ps1.rearrange("p (n c) -> p n c", c=256)[:, :, 128:192])
                pqks.append(pqk)
                pvs.append(pv)
            for wi, init_t in ((0, initQ), (1, initK), (2, initV)):
                psI = ps_w.tile([128, WBS], f32, tag="psW", name="psI")
                for m in range(8):
                    r, mi = m // 4, m % 4
                    pr, h = m // 2, m % 2
                    hs = slice(h * 64, h * 64 + 64)
                    if wi < 2:
                        rhs = pqks[r][:, mi * 128 + wi * 64:
                                      mi * 128 + wi * 64 + 64]
                    else:
                        rhs = pvs[r][:, ts(mi, 64)]
                    lhsW = (WQKh[:, ts(wi, 64)] if wi < 2
                            else W3f[:, ts(2, 64)])
                    nc.tensor.matmul(psI[hs, ts(pr, 64)], lhsW, rhs,
                                     start=True, stop=True)
                rr_copy(init_t[:, ds(g * 4 * 64, WBS)], psI, scale=1.0 / C_NORM)

        if s == 0:
            tap("initQ", initQ)
            tap("initK", initK)
            tap("initV", initV)
        # ---------------- chains ----------------
        flatQ = big.tile([64, M * 64], f16, tag="flatQ", name="flatQ")
        flatK = big.tile([64, M * 64], f16, tag="flatK", name="flatK")
        flatV = big.tile([64, M * 64], f16, tag="flatV", name="flatV")
        for b in range(NBATCH):
            gens = [chain_gen("q", SCHED_QK, DEG_QK, initQ, b, flatQ),
                    chain_gen("k", SCHED_QK, DEG_QK, initK, b, flatK),
                    chain_gen("v", SCHED_V, DEG_V, initV, b, flatV)]
            for _ in range(3):
                next(gens[0], None)
            next(gens[1], None)
            while gens:
                gens = [g for g in gens
                        if next(g, StopIteration) is not StopIteration]

        if s == 0:
            tap("flatQ", flatQ)
            tap("flatK", flatK)
            tap("flatV", flatV)
        # ---------------- attention ----------------
        partQ = work.tile([64, M], f32, tag="partQ", name="partQ")
        partK = work.tile([64, M], f32, tag="partK", name="partK")
        for flat_t, part_t in ((flatQ, partQ), (flatK, partK)):
            sq = big.tile([64, M * 64], f16, tag="sqscr", name="sqscr")
            nc.vector.tensor_mul(sq, flat_t, flat_t)
            nc.vector.tensor_reduce(
                out=part_t, in_=sq.rearrange("p (i c) -> p i c", c=64),
                axis=AX.X, op=OP.add)
        ps_qn = ps_a.tile([1, 64], f32, tag="scrA", name="ps_qn")
        nc.tensor.matmul(ps_qn, ones_col, partQ, start=True, stop=True)
        qn_row = work.tile([1, 64], f32, tag="qnrow", name="qnrow")
        nc.vector.tensor_copy(out=qn_row, in_=ps_qn)
        ps_kn = ps_a.tile([64, 1], f32, tag="scrA", name="ps_kn")
        nc.tensor.matmul(ps_kn, partK, ones_col, start=True, stop=True)
        kn_col = work.tile([64, 1], f32, tag="kncol", name="kncol")
        nc.vector.tensor_copy(out=kn_col, in_=ps_kn)
        ps_qrep = ps_a.tile([64, 64], f32, tag="scrA", name="ps_qrep")
        nc.tensor.matmul(ps_qrep, ones_row, qn_row, start=True, stop=True)
        qrep = work.tile([64, 64], f32, tag="qrep", name="qrep")
        nc.vector.tensor_copy(out=qrep, in_=ps_qrep)

        ps_cross = ps_a.tile([64, 64], f32, tag="scrA", name="ps_cross")
        fQ3 = flatQ.rearrange("p (i c) -> p c i", c=64)
        fK3 = flatK.rearrange("p (i c) -> p c i", c=64)
        for c in range(64):
            nc.tensor.matmul(ps_cross, fK3[:, c, :], fQ3[:, c, :],
                             start=(c == 0), stop=(c == 63))
        Et = work.tile([64, 64], f32, tag="Et", name="Et")
        nc.vector.scalar_tensor_tensor(out=Et, in0=ps_cross, scalar=-2.0,
                                       in1=qrep, op0=OP.mult, op1=OP.add)
        nc.vector.tensor_scalar(out=Et, in0=Et, scalar1=kn_col, scalar2=0.0,
                                op0=OP.add, op1=OP.max)
        lnE = work.tile([64, 64], f32, tag="lnE", name="lnE")
        nc.scalar.activation(out=lnE, in_=Et, func=ACT.Ln,
                             bias=bias_ln, scale=1.0)
        ln1 = work.tile([64, 64], f32, tag="ln1", name="ln1")
        nc.vector.tensor_scalar_add(ln1, lnE, 1.0)
        sc = work.tile([64, 64], f32, tag="sc", name="sc")
        nc.vector.reciprocal(out=sc, in_=ln1)
        expS = work.tile([64, 64], f16, tag="expS", name="expS")
        nc.scalar.activation(out=expS, in_=sc, func=ACT.Exp, bias=0.0,
                             scale=1.0)
        ps_cs = ps_a.tile([64, 1], f32, tag="scrA", name="ps_cs")
        nc.tensor.matmul(ps_cs, expS, ones_col_h, start=True, stop=True)
        inv = work.tile([64, 1], f32, tag="inv", name="inv")
        nc.vector.reciprocal(out=inv, in_=ps_cs)

        # VF: flatV [p, (i c)] -> VF [i, (p c)] via DRAM roundtrip
        VF = big.tile([64, M * 64], f16, tag="VF", name="VF")
        nc.sync.dma_start(out=scrV, in_=flatV)
        nc.sync.dma_start(
            out=VF.rearrange("i (p c) -> i p c", c=64),
            in_=scrV.rearrange("p (i c) -> i p c", c=64))
        if s == 0:
            tap("VF", VF)
        # mixing: M2[j, (p c)] = sum_i expS[i, j] VF[i, (p c)] * inv[j]
        M2 = big.tile([64, M * 64], f16, tag="M2", name="M2")
        for ch in range(4):
            ps_m = ps_b.tile([64, 1024], f32, tag="scrB", name="ps_m")
            nc.tensor.matmul(ps_m[:, 0:512], expS, VF[:, ds(ch * 1024, 512)],
                             start=True, stop=True)
            nc.tensor.matmul(ps_m[:, 512:1024], expS,
                             VF[:, ds(ch * 1024 + 512, 512)],
                             start=True, stop=True)
            nc.vector.tensor_scalar_mul(M2[:, ds(ch * 1024, 1024)], ps_m, inv)
        # S1M scatter: M2 [j=(pr h), (p c)] -> S1M [(h p), (pr c)] via DRAM
        S1M = big.tile([128, NP_PAIR * 64], f16, tag="S1M", name="S1M")
        nc.sync.dma_start(out=scrM, in_=M2)
        for h in (0, 1):
            nc.sync.dma_start(
                out=S1M[h * 64:(h + 1) * 64, :].rearrange(
                    "p (pr c) -> p pr c", c=64),
                in_=scrM.rearrange("(pr two) (p c) -> two p pr c",
                                   two=2, c=64)[h])

        if s == 0:
            tap("M2", M2)
            tap("S1M", S1M)
            tap("expS", expS)
        # ---------------- exp ----------------
        outS1 = big.tile([128, NP_PAIR * 64], f32, tag="outS1", name="outS1")
        for b in range(NBATCH):
            Xs = S1M[:, ds(b * WBS, WBS)]
            X3 = r3(Xs)
            H = chain.tile([128, WBS], f16, tag="expH", name="expH")
            rr_stt(H, Xs, 1.0 / EXP_DEG, get_aI(1.0))
            for k in range(EXP_DEG - 1, 0, -1):
                psx = ps_a.tile([128, WBS], f32, tag="scrA", name="psx")
                px3, h3 = r3(psx), r3(H)
                for hh in (0, 1):
                    hs = slice(hh * 64, hh * 64 + 64)
                    nc.tensor.matmul(psx[hs, :], get_cI(float(k))[hs, :],
                                     get_aI(1.0)[hs, :],
                                     start=True, stop=False)
                for p in range(PB):
                    for hh in (0, 1):
                        hs = slice(hh * 64, hh * 64 + 64)
                        nc.tensor.matmul(px3[hs, p], X3[hs, p], h3[hs, p],
                                         start=False, stop=(p == PB - 1),
                                         skip_group_check=True)
                H2 = chain.tile([128, WBS], f16, tag="expH", name="expH2")
                rr_copy(H2, psx, scale=1.0 / k)
                H = H2
            for sq_i in range(EXP_SQ):
                psx = ps_a.tile([128, WBS], f32, tag="scrA", name="psx2")
                px3, h3 = r3(psx), r3(H)
                for p in range(PB):
                    for hh in (0, 1):
                        hs = slice(hh * 64, hh * 64 + 64)
                        nc.tensor.matmul(px3[hs, p], h3[hs, p], h3[hs, p],
                                         start=True, stop=True)
                if sq_i < EXP_SQ - 1:
                    H2 = chain.tile([128, WBS], f16, tag="expH", name="expH3")
                    rr_copy(H2, psx)
                    H = H2
                else:
                    rr_copy(outS1[:, ds(b * WBS, WBS)], psx, scale=C_NORM)

        o3 = out_ap[s].rearrange("(pr two) r c -> two r pr c", two=2)
        nc.sync.dma_start(
            out=o3[0],
            in_=outS1[0:64, :].rearrange("p (pr c) -> p pr c", c=64))
        nc.sync.dma_start(
            out=o3[1],
            in_=outS1[64:128, :].rearrange("p (pr c) -> p pr c", c=64))


def build(nsamp=NSAMP, num_devices=NCORES, debug_taps=False):
    import concourse.bacc as bacc
    import concourse.mybir as mybir
    import concourse.tile as tile

    nc = bacc.Bacc("TRN2", target_bir_lowering=False, debug=False,
                   num_devices=num_devices)
    f32 = mybir.dt.float32
    x_ap = nc.dram_tensor("x", [nsamp, M, DIN, DIN], f32,
                          kind="ExternalInput").ap()
    wq = nc.dram_tensor("wq", [DIN, DOUT], f32, kind="ExternalInput").ap()
    wk = nc.dram_tensor("wk", [DIN, DOUT], f32, kind="ExternalInput").ap()
    wv = nc.dram_tensor("wv", [DIN, DOUT], f32, kind="ExternalInput").ap()
    out = nc.dram_tensor("out", [nsamp, M, DOUT, DOUT], f32,
                         kind="ExternalOutput").ap()

    taps = {}
    if debug_taps:
        for nm, shp, dt_ in (("initQ", [128, 2048], mybir.dt.float16),
                             ("initK", [128, 2048], mybir.dt.float16),
                             ("initV", [128, 2048], mybir.dt.float16),
                             ("flatQ", [64, 4096], mybir.dt.float16),
                             ("flatK", [64, 4096], mybir.dt.float16),
                             ("flatV", [64, 4096], mybir.dt.float16),
                             ("M2", [64, 4096], mybir.dt.float16),
                             ("VF", [64, 4096], mybir.dt.float16),
                             ("S1M", [128, 2048], mybir.dt.float16),
                             ("expS", [64, 64], mybir.dt.float16)):
            taps[nm] = nc.dram_tensor("tap_" + nm, shp, dt_,
                                      kind="ExternalOutput").ap()
    with tile.TileContext(nc) as tc, ExitStack() as ctx:
        emit_kernel(nc, tc, ctx, x_ap, wq, wk, wv, out, nsamp=nsamp,
                    taps=taps if debug_taps else None)
    nc.compile()
    return nc


_CACHED = {}


def _get_nc(nsamp):
    from concourse.bass_interp import get_hw_module
    if nsamp not in _CACHED:
        nc = build(nsamp=nsamp)
        nc.m = get_hw_module(nc.m)
        _CACHED[nsamp] = nc
    return _CACHED[nsamp]


def kernel(x, Wq, Wk, Wv):
    from concourse.bass_utils import run_bass_kernel_spmd

    bs = x.shape[0]
    nsamp = bs // NCORES
    nc = _get_nc(nsamp)
    in_maps = []
    for c in range(NCORES):
        in_maps.append({
            "x": np.ascontiguousarray(x[c * nsamp:(c + 1) * nsamp],
                                      dtype=np.float32),
            "wq": np.ascontiguousarray(Wq, dtype=np.float32),
            "wk": np.ascontiguousarray(Wk, dtype=np.float32),
            "wv": np.ascontiguousarray(Wv, dtype=np.float32),
        })
    res = run_bass_kernel_spmd(nc, in_maps, list(range(NCORES)))
    outs = [res.results[c]["out"] for c in range(NCORES)]
    full = np.concatenate(outs, axis=0)
    return full.reshape(bs * M, DOUT, DOUT).astype(np.float32)


# revision 27
# speedup vs baseline: 1.0316x; 1.0316x over previous
"""Trainium2 Bass kernel for nn_AttentionManifold (SPD manifold attention).

For each of bs*m=2048 SPD matrices X (100x100): Q/K/V = W^T X W (64x64),
logQ/K/V = matrix log, log-Euclidean attention, mixed = prob-weighted sum
of logV, out = matrix exp(mixed).

Matrix log: Newton-Schulz coupled sqrt chain, L=3 levels, with
first-order level corrections  log A = 2^L log Y_L - sum_l 2^l log W_l,
log W ~= -(I - W);  series log(Y) via deg-8/12 Paterson-Stockmeyer.

Chain scheme '2s_approx' (emulator-validated): exact-transpose pairs
(Y, Yt, Z, Zt) in fp16, P = aI + bW only (no Pt):
    W    = {lhsT=Zt_h, rhs=Y_h}              (per matrix)
    P    = aI + b psW                        (stt)
    Y'   = {lhsT=Yt_h, rhs=P_h}              = Y P
    [Yt'|Z'] = {lhsT=bd(P), rhs=[Yt|Z]}      = [P^T Yt | P^T Z]  (packed)
    Zt'  = {lhsT=Z_h, rhs=P_h}               = (P^T Z)^T bitwise
Every W is a congruence of the SPD input => fp16-stable.

exp via scaling-squaring (deg-4 Horner, 4 squarings).
Sharding: pure data parallelism, bs=32 -> 4 samples per NeuronCore.
"""
import numpy as np
from contextlib import ExitStack

C_NORM = 16.0
BS, M, DIN, DOUT = 32, 64, 100, 64
NCORES = 8
NSAMP = BS // NCORES
NP_PAIR = M // 2          # 32 pairs per sample
PB = 4                    # pairs per chain batch
NBATCH = NP_PAIR // PB    # 8

SCHED_V = [
    [(8.965874126, -13.460097634), (2.380408822, -0.250737931),
     (2.380408822, -0.250532192), (2.380408822, -0.250326648),
     (0.861964497, -0.071654452), (1.542284382, -0.519941516)],
    [(7.758850039, -8.666077201), (0.987610378, -0.093162713),
     (1.645967366, -0.5826622), (1.507505828, -0.502426376)],
    [(6.551825952, -5.22018671), (0.65339645, -0.038866921)],
]
SCHED_QK = [
    [(8.965874126, -13.460097634), (2.380408822, -0.250737931),
     (2.380408822, -0.250532192), (2.380408822, -0.250326648),
     (0.861964497, -0.071654452), (1.542284382, -0.519941516)],
    [(7.758850039, -8.666077201), (0.987610378, -0.093162713),
     (1.645967366, -0.5826622)],
    [(6.404040404, -4.899837718), (0.670769231, -0.04132838)],
]
DEG_QK = 8
DEG_V = 8
EXP_DEG = 4
EXP_SQ = 4
L = 3
WBS = PB * 64             # 256
SBW = PB * 256            # 1024


def emit_kernel(nc, tc, ctx, x_ap, wq_ap, wk_ap, wv_ap, out_ap, nsamp=NSAMP,
                taps=None):
    def tap(name, t):
        if taps is not None and name in taps:
            nc.sync.dma_start(out=taps[name], in_=t)

    import concourse.mybir as mybir
    from concourse.bass import ds, ts
    from concourse.masks import make_identity

    f32 = mybir.dt.float32
    f32r = mybir.dt.float32r
    f16 = mybir.dt.float16
    AX = mybir.AxisListType
    OP = mybir.AluOpType
    ACT = mybir.ActivationFunctionType

    # ---- engine rotation helpers ----
    _rrc = [0]
    _rrs = [0]

    from concourse.bass import MemorySpace as _MS

    def _psum(*aps):
        return any(a.space == _MS.PSUM for a in aps)

    def rr_copy(out, in_, scale=None):
        pool = ((nc.vector, nc.scalar) if _psum(out, in_)
                else (nc.vector, nc.scalar, nc.gpsimd))
        e = pool[_rrc[0] % len(pool)]
        _rrc[0] += 1
        if e is nc.scalar:
            nc.scalar.activation(out=out, in_=in_, func=ACT.Copy, bias=0.0,
                                 scale=1.0 if scale is None else float(scale))
        elif scale is None:
            e.tensor_copy(out=out, in_=in_)
        else:
            e.tensor_scalar_mul(out, in_, float(scale))

    def rr_stt(out, in0, scalar, in1):
        nc.vector.scalar_tensor_tensor(out=out, in0=in0, scalar=float(scalar),
                                       in1=in1, op0=OP.mult, op1=OP.add)

    # ---------------- pools ----------------
    const = ctx.enter_context(tc.tile_pool(name="const", bufs=1))
    work = ctx.enter_context(tc.tile_pool(name="work", bufs=2))
    big = ctx.enter_context(tc.tile_pool(name="big", bufs=1))
    chain = ctx.enter_context(tc.tile_pool(name="chain", bufs=2))
    ps_w = ctx.enter_context(tc.tile_pool(name="ps_w", bufs=1, space="PSUM"))
    ps_bc = ctx.enter_context(tc.tile_pool(name="ps_bc", bufs=2, space="PSUM"))
    ps_a = ctx.enter_context(tc.tile_pool(name="ps_a", bufs=1, space="PSUM"))
    ps_b = ctx.enter_context(tc.tile_pool(name="ps_b", bufs=1, space="PSUM"))

    # ---------------- constants ----------------
    W3f = const.tile([DIN, 3 * DOUT], f32)
    nc.sync.dma_start(out=W3f[:, 0:DOUT], in_=wq_ap)
    nc.sync.dma_start(out=W3f[:, DOUT:2 * DOUT], in_=wk_ap)
    nc.sync.dma_start(out=W3f[:, 2 * DOUT:3 * DOUT], in_=wv_ap)
    W3r = const.tile([DIN, 256], f32r)
    nc.vector.memset(W3r.bitcast(f32), 0.0)
    nc.vector.tensor_copy(out=W3r[:, 0:192], in_=W3f)
    WQKh = const.tile([DIN, 128], f16)        # f16 weights for Q/K stage-2
    nc.vector.tensor_copy(out=WQKh, in_=W3f[:, 0:128])

    IREP = const.tile([128, 64], f16)
    make_identity(nc, IREP[0:64, :])
    make_identity(nc, IREP[64:128, :])

    aI = {}

    def get_aI(val):
        val = float(val)
        if val not in aI:
            t = const.tile([128, WBS], f16, tag=f"aI{len(aI)}",
                           name=f"aI{len(aI)}")
            for p in range(PB):
                nc.vector.tensor_scalar_mul(t[:, ts(p, 64)], IREP, val)
            aI[val] = t
        return aI[val]

    for lv in SCHED_QK + SCHED_V:
        for a, b in lv:
            get_aI(a)
    for c0 in (1.0, 0.25, 0.125):
        get_aI(c0)
    I7f = const.tile([128, WBS], f32)
    for p in range(PB):
        nc.vector.tensor_scalar_mul(I7f[:, ts(p, 64)], IREP, float(2 ** L - 1))
    cI_mm = {}

    def get_cI(val):
        val = float(val)
        if val not in cI_mm:
            t = const.tile([128, 64], f16, tag=f"cImm{len(cI_mm)}",
                           name=f"cImm{len(cI_mm)}")
            nc.vector.tensor_scalar_mul(t, IREP, val)
            cI_mm[val] = t
        return cI_mm[val]

    for v in (1.0, 0.5, 1.0 / 3.0, 2.0, 3.0):
        get_cI(v)
    for lv in SCHED_QK + SCHED_V:
        for a, b in lv:
            get_cI(a / b)
    for v in (0.25, 0.2, 1.0 / 6, 1.0 / 7, 0.125, 1.0 / 9, 0.1,
              1.0 / 11, 1.0 / 12):
        get_cI(v)

    ones_col = const.tile([64, 1], f32)
    nc.vector.memset(ones_col, 1.0)
    ones_col_h = const.tile([64, 1], f16)
    nc.vector.memset(ones_col_h, 16.0)    # folds 1/16 exp prescale into inv
    ones_row = const.tile([1, 64], f32)
    nc.vector.memset(ones_row, 1.0)
    bias_ln = const.tile([64, 1], f32)
    nc.vector.memset(bias_ln, 1.0 + 64e-6)

    # DRAM scratch for partition-moving transposes (DRAM APs unconstrained)
    scrV = nc.dram_tensor("scrV", [64, M * 64], f16, kind="Internal").ap()
    scrM = nc.dram_tensor("scrM", [64, M * 64], f16, kind="Internal").ap()


    def r3(t):
        """[p, (n c)] -> [p, n, 64] view"""
        return t.rearrange("p (n c) -> p n c", c=64)

    def slot(S, f):
        """S [128, (PB,4,64)] -> slot view [128, PB, 64]"""
        return S.rearrange("p (n four c) -> p n four c", four=4, c=64)[:, :, f, :]

    # =====================================================================
    def chain_gen(cn, sched, deg, init_t, b, flat_t):
        """One chain batch: NS chain + corrections + series -> flat_t."""
        ib = r3(init_t[:, ds(b * PB * 64, PB * 64)])     # [128, PB, 64]
        S_cur = None
        Y = Yt = Z = Zt = None       # [128, PB, 64] views
        adj = False
        ACC = None

        for l in range(L):
            steps = sched[l]
            for j, (a, bc) in enumerate(steps):
                Pd = chain.tile([128, WBS], f16, tag=f"Pd{cn}", name=f"Pd{cn}")
                Pd3 = r3(Pd)
                aIt = get_aI(a)
                if j == 0:
                    src = ib if l == 0 else Y
                    rr_stt(Pd3, src, bc, r3(aIt))
                    yield
                else:
                    psW = ps_w.tile([128, WBS], f32, tag="psW", name="psW")
                    psW3 = r3(psW)
                    abI = get_cI(a / bc)
                    Ipat = get_aI(1.0)
                    for h in (0, 1):
                        hs = slice(h * 64, h * 64 + 64)
                        nc.tensor.matmul(psW[hs, :], abI[hs, :], Ipat[hs, :],
                                         start=True, stop=False)
                    for p in range(PB):
                        for h in (0, 1):
                            hs = slice(h * 64, h * 64 + 64)
                            nc.tensor.matmul(psW3[hs, p], Zt[hs, p], Y[hs, p],
                                             start=False,
                                             stop=(p == PB - 1),
                                             skip_group_check=True)
                    yield
                    nc.vector.tensor_scalar_mul(Pd, psW, float(bc))
                    yield
                # ---- updates ----
                psBC = ps_bc.tile([128, SBW], f32, tag="psBC", name="psBC")
                ps4 = psBC.rearrange("p (n four c) -> p n four c", four=4, c=64)
                YtP = ib if (l == 0 and j == 0) else Yt
                for p in range(PB):
                    for h in (0, 1):
                        hs = slice(h * 64, h * 64 + 64)
                        # Y' = Yt^T P ; Yt' = P^T Yt
                        nc.tensor.matmul(ps4[hs, p, 0, :], YtP[hs, p],
                                         Pd3[hs, p], start=True, stop=True)
                        nc.tensor.matmul(ps4[hs, p, 1, :], Pd3[hs, p],
                                         YtP[hs, p], start=True, stop=True)
                        if j > 0:
                            # Z' = P^T Z ; Zt' = Z^T P
                            nc.tensor.matmul(ps4[hs, p, 2, :], Pd3[hs, p],
                                             Z[hs, p], start=True, stop=True)
                            nc.tensor.matmul(ps4[hs, p, 3, :], Z[hs, p],
                                             Pd3[hs, p], start=True, stop=True)
                yield
                S_new = chain.tile([128, SBW], f16, tag=f"S{cn}", name=f"S{cn}")
                s4 = S_new.rearrange("p (n four c) -> p n four c", four=4, c=64)
                if j == 0:
                    nc.vector.tensor_copy(out=s4[:, :, 0:2, :],
                                          in_=ps4[:, :, 0:2, :])
                    Z = Pd3
                    Zt = Pd3
                    adj = False
                else:
                    nc.scalar.activation(out=S_new[:, 0:SBW * 3 // 4],
                                         in_=psBC[:, 0:SBW * 3 // 4],
                                         func=ACT.Copy, bias=0.0, scale=1.0)
                    nc.vector.tensor_copy(out=S_new[:, SBW * 3 // 4:],
                                          in_=psBC[:, SBW * 3 // 4:])
                    Z = slot(S_new, 2)
                    Zt = slot(S_new, 3)
                    adj = True
                Y = slot(S_new, 0)
                Yt = slot(S_new, 1)
                S_cur = S_new
                yield
            # ---- level end correction: psWe = Zt^T Y + Y^T Zt ----
            psWe = ps_w.tile([128, WBS], f32, tag="psW", name="psWe")
            pw3 = r3(psWe)
            for p in range(PB):
                for h in (0, 1):
                    hs = slice(h * 64, h * 64 + 64)
                    nc.tensor.matmul(pw3[hs, p], Zt[hs, p], Y[hs, p],
                                     start=True, stop=False)
                    nc.tensor.matmul(pw3[hs, p], Y[hs, p], Zt[hs, p],
                                     start=False, stop=True)
            yield
            if l == 0:
                ACC = chain.tile([128, WBS], f32, tag=f"acc{cn}",
                                 name=f"acc{cn}")
                rr_stt(ACC, psWe, -0.5, I7f)
            else:
                rr_stt(ACC, psWe, -float(2 ** l) / 2.0, ACC)
            yield
        # ================= series =================
        E = chain.tile([128, WBS], f16, tag=f"E{cn}", name=f"E{cn}")
        rr_stt(r3(E), Y, -1.0, r3(get_aI(1.0)))
        yield
        powers = {1: E}
        for k, rt in ((2, 1), (3, 2), (4, 3)):
            psE = ps_a.tile([128, WBS], f32, tag="scrA", name="psE")
            pe3 = r3(psE)
            e1 = r3(powers[1])
            ert = r3(powers[rt])
            for p in range(PB):
                for h in (0, 1):
                    hs = slice(h * 64, h * 64 + 64)
                    nc.tensor.matmul(pe3[hs, p], e1[hs, p], ert[hs, p],
                                     start=True, stop=True)
            Ek = chain.tile([128, WBS], f16, tag=f"E{k}{cn}", name=f"E{k}{cn}")
            rr_copy(Ek, psE)
            powers[k] = Ek
            yield
        E2, E3, E4 = powers[2], powers[3], powers[4]

        def combo(coefs, dst_tag):
            """PE-accumulated c0 I + c1 E + c2 E2 + c3 E3 + c4 E4 -> f16."""
            psC = ps_a.tile([128, WBS], f32, tag="scrA", name="psC")
            ops = [(get_cI(coefs[0]), get_aI(1.0))] + [
                (get_cI(cv), pw) for cv, pw in
                zip(coefs[1:], (E, E2, E3, E4)) if cv]
            for i, (lh, rh) in enumerate(ops):
                for h in (0, 1):
                    hs = slice(h * 64, h * 64 + 64)
                    nc.tensor.matmul(psC[hs, :], lh[hs, :], rh[hs, :],
                                     start=(i == 0),
                                     stop=(i == len(ops) - 1),
                                     skip_group_check=(i > 0))
            Ct = chain.tile([128, WBS], f16, tag=dst_tag, name=dst_tag)
            rr_copy(Ct, psC)
            return Ct

        C = combo((0.25, 0.2, 1.0 / 6, 1.0 / 7, 0.125), f"C{cn}")
        yield
        if deg == 12:
            C2 = combo((0.125, 1.0 / 9, 0.1, 1.0 / 11, 1.0 / 12), f"C2{cn}")
            psH = ps_a.tile([128, WBS], f32, tag="scrA", name="psH")
            ph3, e43, c23 = r3(psH), r3(E4), r3(C2)
            for p in range(PB):
                for h in (0, 1):
                    hs = slice(h * 64, h * 64 + 64)
                    nc.tensor.matmul(ph3[hs, p], e43[hs, p], c23[hs, p],
                                     start=True, stop=True)
            yield
            CH = chain.tile([128, WBS], f16, tag=f"C2{cn}", name=f"CH{cn}")
            nc.vector.tensor_tensor(out=CH, in0=psH, in1=C, op=OP.add)
            C = CH
            yield
        # psB0 = 1*E + E4@C + (1/2)E2 + (1/3)E3   (accumulated group)
        psB0 = ps_a.tile([128, WBS], f32, tag="scrA", name="psB0")
        e43, c3 = r3(E4), r3(C)
        for h in (0, 1):
            hs = slice(h * 64, h * 64 + 64)
            nc.tensor.matmul(psB0[hs, :], get_cI(1.0)[hs, :], E[hs, :],
                             start=True, stop=False)
        for p in range(PB):
            for h in (0, 1):
                hs = slice(h * 64, h * 64 + 64)
                nc.tensor.matmul(r3(psB0)[hs, p], e43[hs, p], c3[hs, p],
                                 start=False, stop=False,
                                 skip_group_check=True)
        for h in (0, 1):
            hs = slice(h * 64, h * 64 + 64)
            nc.tensor.matmul(psB0[hs, :], get_cI(0.5)[hs, :], E2[hs, :],
                             start=False, stop=False, skip_group_check=True)
            nc.tensor.matmul(psB0[hs, :], get_cI(1.0 / 3.0)[hs, :], E3[hs, :],
                             start=False, stop=True, skip_group_check=True)
        yield
        # LS = -2^L psB0 + ACC -> flat (strided, per h)
        fl3 = flat_t.rearrange("p (pr two c) -> p pr two c", two=2, c=64)
        acc3 = r3(ACC)
        for h in (0, 1):
            hs = slice(h * 64, h * 64 + 64)
            rr_stt(fl3[:, ds(b * PB, PB), h, :], r3(psB0)[hs], -float(2 ** L),
                   acc3[hs])
        yield

    # ======================= per-sample pipeline =========================
    for s in range(nsamp):
        initQ = work.tile([128, NP_PAIR * 64], f16, tag="initQ", name="initQ")
        initK = work.tile([128, NP_PAIR * 64], f16, tag="initK", name="initK")
        initV = work.tile([128, NP_PAIR * 64], f16, tag="initV", name="initV")

        # ---------------- congruence ----------------
        for g in range(8):          # 8 matrices per group
            if g % 2 == 0:
                xbuf = work.tile([DIN, 16 * DIN], f32r, tag="xbuf", name="xbuf")
                nc.gpsimd.dma_start(
                    out=xbuf.rearrange("p (i c) -> p i c", c=DIN),
                    in_=x_ap[s, ds(g * 8, 16)].rearrange("i p c -> p i c"))
            pqks = []
            pvs = []
            for r in range(2):      # 2 rounds x 4 matrices
                ps1 = ps_b.tile([DIN, 4 * 256], f32, tag="scrB", name="ps1")
                for mi in range(4):
                    mg = (g % 2) * 8 + r * 4 + mi
                    nc.tensor.matmul(ps1[:, ts(mi, 256)],
                                     xbuf[:, ts(mg, DIN)], W3r,
                                     start=True, stop=True)
                pqk = work.tile([DIN, 4 * 128], f16, tag="pqk", name="pqk")
                pv = work.tile([DIN, 4 * 64], f32, tag="pv", name="pv")
                rr_copy(pqk.rearrange("p (n c) -> p n c", c=128),
                        ps1.rearrange("p (n c) -> p n c", c=256)[:, :, 0:128])
                rr_copy(pv.rearrange("p (n c) -> p n c", c=64),
                        ps1.rearrange("p (n c) -> p n c", c=256)[:, :, 128:192])
                pqks.append(pqk)
                pvs.append(pv)
            for wi, init_t in ((0, initQ), (1, initK), (2, initV)):
                psI = ps_w.tile([128, WBS], f32, tag="psW", name="psI")
                for m in range(8):
                    r, mi = m // 4, m % 4
                    pr, h = m // 2, m % 2
                    hs = slice(h * 64, h * 64 + 64)
                    if wi < 2:
                        rhs = pqks[r][:, mi * 128 + wi * 64:
                                      mi * 128 + wi * 64 + 64]
                    else:
                        rhs = pvs[r][:, ts(mi, 64)]
                    lhsW = (WQKh[:, ts(wi, 64)] if wi < 2
                            else W3f[:, ts(2, 64)])
                    nc.tensor.matmul(psI[hs, ts(pr, 64)], lhsW, rhs,
                                     start=True, stop=True)
                rr_copy(init_t[:, ds(g * 4 * 64, WBS)], psI, scale=1.0 / C_NORM)

        if s == 0:
            tap("initQ", initQ)
            tap("initK", initK)
            tap("initV", initV)
        # ---------------- chains ----------------
        flatQ = big.tile([64, M * 64], f16, tag="flatQ", name="flatQ")
        flatK = big.tile([64, M * 64], f16, tag="flatK", name="flatK")
        flatV = big.tile([64, M * 64], f16, tag="flatV", name="flatV")
        def stream(cn, sch, dg, it_, fl):
            # fuse all batches of one chain into a single generator: no
            # batch-boundary barrier (V-tail no longer runs alone)
            for b in range(NBATCH):
                yield from chain_gen(cn, sch, dg, it_, b, fl)

        streams = [stream("q", SCHED_QK, DEG_QK, initQ, flatQ),
                   stream("k", SCHED_QK, DEG_QK, initK, flatK),
                   stream("v", SCHED_V, DEG_V, initV, flatV)]
        next(streams[0], None)
        next(streams[0], None)
        next(streams[1], None)
        while streams:
            streams = [g for g in streams
                       if next(g, StopIteration) is not StopIteration]

        if s == 0:
            tap("flatQ", flatQ)
            tap("flatK", flatK)
            tap("flatV", flatV)
        # ---------------- attention ----------------
        partQ = work.tile([64, M], f32, tag="partQ", name="partQ")
        partK = work.tile([64, M], f32, tag="partK", name="partK")
        for flat_t, part_t in ((flatQ, partQ), (flatK, partK)):
            sq = big.tile([64, M * 64], f16, tag="sqscr", name="sqscr")
            nc.vector.tensor_mul(sq, flat_t, flat_t)
            nc.vector.tensor_reduce(
                out=part_t, in_=sq.rearrange("p (i c) -> p i c", c=64),
                axis=AX.X, op=OP.add)
        ps_qn = ps_a.tile([1, 64], f32, tag="scrA", name="ps_qn")
        nc.tensor.matmul(ps_qn, ones_col, partQ, start=True, stop=True)
        qn_row = work.tile([1, 64], f32, tag="qnrow", name="qnrow")
        nc.vector.tensor_copy(out=qn_row, in_=ps_qn)
        ps_kn = ps_a.tile([64, 1], f32, tag="scrA", name="ps_kn")
        nc.tensor.matmul(ps_kn, partK, ones_col, start=True, stop=True)
        kn_col = work.tile([64, 1], f32, tag="kncol", name="kncol")
        nc.vector.tensor_copy(out=kn_col, in_=ps_kn)
        ps_qrep = ps_a.tile([64, 64], f32, tag="scrA", name="ps_qrep")
        nc.tensor.matmul(ps_qrep, ones_row, qn_row, start=True, stop=True)
        qrep = work.tile([64, 64], f32, tag="qrep", name="qrep")
        nc.vector.tensor_copy(out=qrep, in_=ps_qrep)

        ps_cross = ps_a.tile([64, 64], f32, tag="scrA", name="ps_cross")
        fQ3 = flatQ.rearrange("p (i c) -> p c i", c=64)
        fK3 = flatK.rearrange("p (i c) -> p c i", c=64)
        for c in range(64):
            nc.tensor.matmul(ps_cross, fK3[:, c, :], fQ3[:, c, :],
                             start=(c == 0), stop=(c == 63))
        Et = work.tile([64, 64], f32, tag="Et", name="Et")
        nc.vector.scalar_tensor_tensor(out=Et, in0=ps_cross, scalar=-2.0,
                                       in1=qrep, op0=OP.mult, op1=OP.add)
        nc.vector.tensor_scalar(out=Et, in0=Et, scalar1=kn_col, scalar2=0.0,
                                op0=OP.add, op1=OP.max)
        lnE = work.tile([64, 64], f32, tag="lnE", name="lnE")
        nc.scalar.activation(out=lnE, in_=Et, func=ACT.Ln,
                             bias=bias_ln, scale=1.0)
        ln1 = work.tile([64, 64], f32, tag="ln1", name="ln1")
        nc.vector.tensor_scalar_add(ln1, lnE, 1.0)
        sc = work.tile([64, 64], f32, tag="sc", name="sc")
        nc.vector.reciprocal(out=sc, in_=ln1)
        expS = work.tile([64, 64], f16, tag="expS", name="expS")
        nc.scalar.activation(out=expS, in_=sc, func=ACT.Exp, bias=0.0,
                             scale=1.0)
        ps_cs = ps_a.tile([64, 1], f32, tag="scrA", name="ps_cs")
        nc.tensor.matmul(ps_cs, expS, ones_col_h, start=True, stop=True)
        inv = work.tile([64, 1], f32, tag="inv", name="inv")
        nc.vector.reciprocal(out=inv, in_=ps_cs)

        # VF: flatV [p, (i c)] -> VF [i, (p c)] via DRAM roundtrip
        VF = big.tile([64, M * 64], f16, tag="VF", name="VF")
        nc.sync.dma_start(out=scrV, in_=flatV)
        nc.sync.dma_start(
            out=VF.rearrange("i (p c) -> i p c", c=64),
            in_=scrV.rearrange("p (i c) -> i p c", c=64))
        if s == 0:
            tap("VF", VF)
        # mixing: M2[j, (p c)] = sum_i expS[i, j] VF[i, (p c)] * inv[j]
        M2 = big.tile([64, M * 64], f16, tag="M2", name="M2")
        for ch in range(4):
            ps_m = ps_b.tile([64, 1024], f32, tag="scrB", name="ps_m")
            nc.tensor.matmul(ps_m[:, 0:512], expS, VF[:, ds(ch * 1024, 512)],
                             start=True, stop=True)
            nc.tensor.matmul(ps_m[:, 512:1024], expS,
                             VF[:, ds(ch * 1024 + 512, 512)],
                             start=True, stop=True)
            nc.vector.tensor_scalar_mul(M2[:, ds(ch * 1024, 1024)], ps_m, inv)
        # S1M scatter: M2 [j=(pr h), (p c)] -> S1M [(h p), (pr c)] via DRAM
        S1M = big.tile([128, NP_PAIR * 64], f16, tag="S1M", name="S1M")
        nc.sync.dma_start(out=scrM, in_=M2)
        for h in (0, 1):
            nc.sync.dma_start(
                out=S1M[h * 64:(h + 1) * 64, :].rearrange(
                    "p (pr c) -> p pr c", c=64),
                in_=scrM.rearrange("(pr two) (p c) -> two p pr c",
                                   two=2, c=64)[h])

        if s == 0:
            tap("M2", M2)
            tap("S1M", S1M)
            tap("expS", expS)
        # ---------------- exp ----------------
        outS1 = big.tile([128, NP_PAIR * 64], f32, tag="outS1", name="outS1")
        for b in range(NBATCH):
            Xs = S1M[:, ds(b * WBS, WBS)]
            X3 = r3(Xs)
            H = chain.tile([128, WBS], f16, tag="expH", name="expH")
            rr_stt(H, Xs, 1.0 / EXP_DEG, get_aI(1.0))
            for k in range(EXP_DEG - 1, 0, -1):
                psx = ps_a.tile([128, WBS], f32, tag="scrA", name="psx")
                px3, h3 = r3(psx), r3(H)
                for hh in (0, 1):
                    hs = slice(hh * 64, hh * 64 + 64)
                    nc.tensor.matmul(psx[hs, :], get_cI(float(k))[hs, :],
                                     get_aI(1.0)[hs, :],
                                     start=True, stop=False)
                for p in range(PB):
                    for hh in (0, 1):
                        hs = slice(hh * 64, hh * 64 + 64)
                        nc.tensor.matmul(px3[hs, p], X3[hs, p], h3[hs, p],
                                         start=False, stop=(p == PB - 1),
                                         skip_group_check=True)
                H2 = chain.tile([128, WBS], f16, tag="expH", name="expH2")
                rr_copy(H2, psx, scale=1.0 / k)
                H = H2
            for sq_i in range(EXP_SQ):
                psx = ps_a.tile([128, WBS], f32, tag="scrA", name="psx2")
                px3, h3 = r3(psx), r3(H)
                for p in range(PB):
                    for hh in (0, 1):
                        hs = slice(hh * 64, hh * 64 + 64)
                        nc.tensor.matmul(px3[hs, p], h3[hs, p], h3[hs, p],
                                         start=True, stop=True)
                if sq_i < EXP_SQ - 1:
                    H2 = chain.tile([128, WBS], f16, tag="expH", name="expH3")
                    rr_copy(H2, psx)
                    H = H2
                else:
                    rr_copy(outS1[:, ds(b * WBS, WBS)], psx, scale=C_NORM)

        o3 = out_ap[s].rearrange("(pr two) r c -> two r pr c", two=2)
        nc.sync.dma_start(
            out=o3[0],
            in_=outS1[0:64, :].rearrange("p (pr c) -> p pr c", c=64))
        nc.sync.dma_start(
            out=o3[1],
            in_=outS1[64:128, :].rearrange("p (pr c) -> p pr c", c=64))


def build(nsamp=NSAMP, num_devices=NCORES, debug_taps=False):
    import concourse.bacc as bacc
    import concourse.mybir as mybir
    import concourse.tile as tile

    nc = bacc.Bacc("TRN2", target_bir_lowering=False, debug=False,
                   num_devices=num_devices)
    f32 = mybir.dt.float32
    x_ap = nc.dram_tensor("x", [nsamp, M, DIN, DIN], f32,
                          kind="ExternalInput").ap()
    wq = nc.dram_tensor("wq", [DIN, DOUT], f32, kind="ExternalInput").ap()
    wk = nc.dram_tensor("wk", [DIN, DOUT], f32, kind="ExternalInput").ap()
    wv = nc.dram_tensor("wv", [DIN, DOUT], f32, kind="ExternalInput").ap()
    out = nc.dram_tensor("out", [nsamp, M, DOUT, DOUT], f32,
                         kind="ExternalOutput").ap()

    taps = {}
    if debug_taps:
        for nm, shp, dt_ in (("initQ", [128, 2048], mybir.dt.float16),
                             ("initK", [128, 2048], mybir.dt.float16),
                             ("initV", [128, 2048], mybir.dt.float16),
                             ("flatQ", [64, 4096], mybir.dt.float16),
                             ("flatK", [64, 4096], mybir.dt.float16),
                             ("flatV", [64, 4096], mybir.dt.float16),
                             ("M2", [64, 4096], mybir.dt.float16),
                             ("VF", [64, 4096], mybir.dt.float16),
                             ("S1M", [128, 2048], mybir.dt.float16),
                             ("expS", [64, 64], mybir.dt.float16)):
            taps[nm] = nc.dram_tensor("tap_" + nm, shp, dt_,
                                      kind="ExternalOutput").ap()
    with tile.TileContext(nc) as tc, ExitStack() as ctx:
        emit_kernel(nc, tc, ctx, x_ap, wq, wk, wv, out, nsamp=nsamp,
                    taps=taps if debug_taps else None)
    nc.compile()
    return nc


_CACHED = {}


def _get_nc(nsamp):
    from concourse.bass_interp import get_hw_module
    if nsamp not in _CACHED:
        nc = build(nsamp=nsamp)
        nc.m = get_hw_module(nc.m)
        _CACHED[nsamp] = nc
    return _CACHED[nsamp]


def kernel(x, Wq, Wk, Wv):
    from concourse.bass_utils import run_bass_kernel_spmd

    bs = x.shape[0]
    nsamp = bs // NCORES
    nc = _get_nc(nsamp)
    in_maps = []
    for c in range(NCORES):
        in_maps.append({
            "x": np.ascontiguousarray(x[c * nsamp:(c + 1) * nsamp],
                                      dtype=np.float32),
            "wq": np.ascontiguousarray(Wq, dtype=np.float32),
            "wk": np.ascontiguousarray(Wk, dtype=np.float32),
            "wv": np.ascontiguousarray(Wv, dtype=np.float32),
        })
    res = run_bass_kernel_spmd(nc, in_maps, list(range(NCORES)))
    outs = [res.results[c]["out"] for c in range(NCORES)]
    full = np.concatenate(outs, axis=0)
    return full.reshape(bs * M, DOUT, DOUT).astype(np.float32)
